# revision 26
# baseline (speedup 1.0000x reference)
"""RBF kernel ridge regression inference on 8 Trainium2 NeuronCores.

out[q] = sum_t exp(-gamma * ||X[q] - T[t]||^2) * coef[t]

Factored as sum_t (coef[t] * exp(-g*y2[t])) * exp(2g*dot[q,t] - g*x2[q]):
the train-side norm folds into the coefficients on the host, the query-side
norm folds into the ScalarE activation bias.  On device each core runs only
three pipelined stages per (query-chunk, train-segment) tile:

  TensorE : fp8e4 DoubleRow GEMM (queries stationary)  -> psum [128q, W]
  ScalarE : one wide Exp over the psum banks, bias=-g*x2 -> et bf16
  VectorE : et * coef' multiply-accumulate along free  -> partial out [128,1]

Train columns are processed in segments (ramping 1024 -> 2048 wide so the
first matmul only waits for ~1MB of DMA).  All device buffers are laid out
segment-major so every DMA is contiguous per partition (2-8KB descriptors).
Queries are sharded across the 8 cores; train data and coefficients are
replicated.
"""

import numpy as np
import ml_dtypes

GAMMA = 1.0
N_QUERY, N_TRAIN, D = 8192, 8192, 512
N_CORES = 8
P = 128
QPC = N_QUERY // N_CORES    # 1024 queries per core
KD = D // P                 # 4 contraction subtiles of 128
NQC = QPC // P              # 8 query chunks of 128 (stationary side)
TT = 512                    # one psum bank / one matmul free dim
SEGW = [512, 1536, 2048, 2048, 2048]    # train-column segment widths
SEGO = [sum(SEGW[:i]) for i in range(len(SEGW))]
NSEG = len(SEGW)

_CACHE = {}


def _build_program(probe=False):
    from contextlib import ExitStack

    import concourse.bass as bass
    import concourse.mybir as mybir
    import concourse.tile as tile
    from concourse import bacc

    f32 = mybir.dt.float32
    bf16 = mybir.dt.bfloat16
    fp8 = mybir.dt.float8e4
    AF = mybir.ActivationFunctionType
    MUL = mybir.AluOpType.mult
    ADD = mybir.AluOpType.add
    DR = mybir.MatmulPerfMode.DoubleRow

    nc = bacc.Bacc(
        "TRN2", target_bir_lowering=False, debug=False, num_devices=N_CORES
    )

    tt_d = nc.dram_tensor("tt_fp8", [P, KD * N_TRAIN], fp8, kind="ExternalInput").ap()
    xt_d = nc.dram_tensor("xt_fp8", [P, NQC * KD, P], fp8, kind="ExternalInput").ap()
    coef_d = nc.dram_tensor("coef_rep", [P, N_TRAIN], bf16, kind="ExternalInput").ap()
    x2_d = nc.dram_tensor("x2neg", [P, NQC], f32, kind="ExternalInput").ap()
    # natural [partition, qc] layout — host transposes; a "(c p) -> p c"
    # rearrange here would emit 1024 scattered 4B DMA writes (~11us tail)
    out_d = nc.dram_tensor("out", [P, NQC], f32, kind="ExternalOutput").ap()

    with tile.TileContext(nc) as tc, ExitStack() as ctx:
        res = ctx.enter_context(tc.tile_pool(name="res", bufs=1))
        etp = ctx.enter_context(tc.tile_pool(name="etp", bufs=3))
        scrp = ctx.enter_context(tc.tile_pool(name="scrp", bufs=2))
        psq = ctx.enter_context(tc.tile_pool(name="psq", bufs=2, space="PSUM"))

        # warm the ScalarE activation tables immediately: the implicit
        # ACT_TABLE_LOAD (~2.7us) binds before the first real Exp otherwise
        dum = res.tile([1, 1], f32, tag="dum")
        nc.vector.memset(dum[:], 0.0)
        dum2 = res.tile([1, 1], f32, tag="dum2")
        nc.scalar.activation(dum2[:], dum[:], AF.Exp)

        # ---- prologue: resident loads, segment-major so compute overlaps ----
        coef_sb = res.tile([P, N_TRAIN], bf16, tag="coef")
        xt_sb = res.tile([P, NQC * KD, P], fp8, tag="xt")
        half = NQC * KD // 2
        tts = []
        for s in range(NSEG):
            w = SEGW[s]
            t = res.tile([P, KD, w], fp8, tag=f"tt{s}", name=f"tt{s}")
            nc.sync.dma_start(
                t[:], tt_d[:, KD * SEGO[s] : KD * (SEGO[s] + w)].rearrange(
                    "p (k w) -> p k w", k=KD
                )
            )
            tts.append(t)
            if s == 0:
                nc.sync.dma_start(xt_sb[:, :KD, :], xt_d[:, :KD, :])
                nc.sync.dma_start(
                    coef_sb[:, : SEGW[0]], coef_d[:, : SEGW[0]]
                )
                x2c = res.tile([P, NQC], f32, tag="x2c")
                nc.sync.dma_start(x2c[:], x2_d[:])
                nc.sync.dma_start(xt_sb[:, KD:half, :], xt_d[:, KD:half, :])
                nc.sync.dma_start(xt_sb[:, half:, :], xt_d[:, half:, :])
            else:
                nc.sync.dma_start(
                    coef_sb[:, SEGO[s] : SEGO[s] + w],
                    coef_d[:, SEGO[s] : SEGO[s] + w],
                )

        pacc = res.tile([P, NQC * NSEG], f32, tag="pacc")
        outc = res.tile([P, NQC], f32, tag="outc")

        # warm the PE's HAM clock gate (3.4us of sustained activity lifts it
        # from 1.2 to 2.4GHz) with two fp32 dummy matmuls (4 cyc/row) while
        # the first DMAs are still in flight; real matmuls then start warm
        dmov = res.tile([1, 512], f32, tag="dmov")
        nc.vector.memset(dmov[:], 0.0)
        ps_warm = psq.tile([P, 4 * SEGW[0]], f32, tag="ps", name="ps_warm")
        for i in range(2):
            nc.tensor.matmul(
                ps_warm[0:1, i * 512 : (i + 1) * 512], dum[:], dmov[:],
                start=True, stop=True,
            )

        # ---- main loop: train segments x 8 query chunks ----
        # interleave the first s1 iterations into s0's tail so the DVE has
        # ready tiles across the segment seam (s1's wider exp lags behind)
        order = [(s, qc) for s in range(NSEG) for qc in range(NQC)]
        order.remove((1, 0))
        order.remove((1, 1))
        order.insert(order.index((0, 4)), (1, 0))
        order.insert(order.index((0, 6)), (1, 1))
        for s, qc in order:
            w = SEGW[s]
            if True:
                ps = psq.tile([P, w], f32, tag="ps")
                for kd2 in range(KD // 2):
                    wst = xt_sb[:, qc * KD + 2 * kd2 : qc * KD + 2 * kd2 + 2, :]
                    for k in range(w // TT):
                        nc.tensor.matmul(
                            ps[:, k * TT : (k + 1) * TT],
                            wst,
                            tts[s][:, 2 * kd2 : 2 * kd2 + 2, k * TT : (k + 1) * TT],
                            start=(kd2 == 0),
                            stop=(kd2 == KD // 2 - 1),
                            perf_mode=DR,
                        )
                et = etp.tile([P, w], bf16, tag="et")
                nc.scalar.activation(
                    et[:], ps[:], AF.Exp, bias=x2c[:, qc : qc + 1], scale=2.0 * GAMMA
                )
                scr = scrp.tile([P, w], bf16, tag="scr")
                nc.vector.scalar_tensor_tensor(
                    scr[:], et[:], 1.0, coef_sb[:, SEGO[s] : SEGO[s] + w], MUL, MUL,
                    accum_out=pacc[:, qc * NSEG + s : qc * NSEG + s + 1],
                )
                if s == NSEG - 1:
                    # all partials for this qc are in; fold them right away
                    nc.vector.tensor_reduce(
                        outc[:, qc : qc + 1],
                        pacc[:, qc * NSEG : (qc + 1) * NSEG],
                        mybir.AxisListType.X,
                        ADD,
                    )
        nc.sync.dma_start(out_d, outc[:])

        if probe:
            # one-shot DVE perf-mode probes (read from the trace, then drop)
            prb_mul = res.tile([P, 2048], bf16, tag="prbm")
            nc.vector.tensor_mul(
                prb_mul[:], coef_sb[:, 0:2048], coef_sb[:, 2048:4096]
            )
            prb_ts = res.tile([P, 2048], bf16, tag="prbt")
            prb_acc = res.tile([P, 1], f32, tag="prba")
            nc.vector.tensor_scalar(
                prb_ts[:], prb_mul[:], 1.0, 0.0, MUL, ADD, accum_out=prb_acc[:]
            )

    nc.compile()
    return nc


def _get_program():
    if "nc" not in _CACHE:
        _CACHE["nc"] = _build_program()
    return _CACHE["nc"]


def make_in_maps(X, train_X, dual_coef):
    fp8 = ml_dtypes.float8_e4m3
    bf = ml_dtypes.bfloat16

    # train side, segment-major: per segment [P, KD*w] contiguous
    segs = []
    for s in range(NSEG):
        t0, w = SEGO[s], SEGW[s]
        seg = train_X[t0 : t0 + w].T.reshape(KD, P, w).transpose(1, 0, 2)
        segs.append(seg.reshape(P, KD * w))
    ttb = np.ascontiguousarray(np.concatenate(segs, axis=1)).astype(fp8)

    # fold exp(-g*||T_t||^2) into the coefficients, replicate across partitions
    y2 = np.sum(train_X.astype(np.float32) ** 2, axis=1)
    coef_f = (dual_coef.astype(np.float32) * np.exp(-GAMMA * y2)).astype(bf)
    coefb = np.ascontiguousarray(np.broadcast_to(coef_f[None, :], (P, N_TRAIN)))

    in_maps = []
    for c in range(N_CORES):
        Xc = X[c * QPC : (c + 1) * QPC]
        # [p, qc*KD+kd, j] = Xc[qc*128+j, kd*128+p]
        xtb = np.ascontiguousarray(
            Xc.reshape(NQC, P, KD, P).transpose(3, 0, 2, 1).reshape(P, NQC * KD, P)
        ).astype(fp8)
        x2 = np.sum(Xc.astype(np.float32) ** 2, axis=1)
        x2neg = np.ascontiguousarray((-GAMMA * x2).reshape(NQC, P).T)
        in_maps.append(
            {
                "tt_fp8": ttb,
                "xt_fp8": xtb,
                "coef_rep": coefb,
                "x2neg": x2neg,
            }
        )
    return in_maps


def _get_callable():
    """Cached (fn, in_names, out_names, out_avals, zero_outs, mesh) for the
    sharded 8-core NEFF execution."""
    if "call" in _CACHE:
        return _CACHE["call"]

    import jax
    from jax.sharding import Mesh, PartitionSpec
    from jax.experimental.shard_map import shard_map

    import concourse.mybir as mybir
    from concourse import bass2jax
    from concourse.bass2jax import install_neuronx_cc_hook

    install_neuronx_cc_hook()
    nc = _get_program()

    partition_name = (
        nc.partition_id_tensor.name if nc.partition_id_tensor else None
    )
    in_names, out_names, out_avals, zero_outs = [], [], [], []
    for alloc in nc.m.functions[0].allocations:
        if not isinstance(alloc, mybir.MemoryLocationSet):
            continue
        if alloc.kind not in ("ExternalInput", "ExternalOutput"):
            continue
        name = alloc.memorylocations[0].name
        if alloc.kind == "ExternalInput":
            if name != partition_name:
                in_names.append(name)
        else:
            out_names.append(name)
            shape = tuple(alloc.tensor_shape)
            dtype = mybir.dt.np(alloc.dtype)
            out_avals.append(jax.core.ShapedArray(shape, dtype))
            zero_outs.append(np.zeros(shape, dtype))
    all_in_names = in_names + out_names
    if partition_name is not None:
        all_in_names = all_in_names + [partition_name]

    def _body(*args):
        operands = list(args)
        if partition_name is not None:
            operands.append(bass2jax.partition_id_tensor())
        outs = bass2jax._bass_exec_p.bind(
            *operands,
            out_avals=tuple(out_avals),
            in_names=tuple(all_in_names),
            out_names=tuple(out_names),
            lowering_input_output_aliases=(),
            sim_require_finite=True,
            sim_require_nnan=True,
            nc=nc,
        )
        return tuple(outs)

    devices = jax.devices()[:N_CORES]
    mesh = Mesh(np.asarray(devices), ("core",))
    n_all = len(in_names) + len(out_names)
    fn = jax.jit(
        shard_map(
            _body,
            mesh=mesh,
            in_specs=(PartitionSpec("core"),) * n_all,
            out_specs=(PartitionSpec("core"),) * len(out_names),
            check_rep=False,
        ),
        keep_unused=True,
    )
    _CACHE["call"] = (fn, in_names, out_names, out_avals, zero_outs, mesh)
    return _CACHE["call"]


def concat_inputs(in_maps):
    fn, in_names, out_names, out_avals, zero_outs, mesh = _get_callable()
    concat_in = [
        np.concatenate([np.asarray(m[name]) for m in in_maps], axis=0)
        for name in in_names
    ]
    concat_zeros = [
        np.zeros((N_CORES * z.shape[0], *z.shape[1:]), z.dtype) for z in zero_outs
    ]
    return concat_in + concat_zeros


def kernel(X, train_X, dual_coef):
    X = np.asarray(X, dtype=np.float32)
    train_X = np.asarray(train_X, dtype=np.float32)
    dual_coef = np.asarray(dual_coef, dtype=np.float32)

    fn, in_names, out_names, out_avals, zero_outs, mesh = _get_callable()
    in_maps = make_in_maps(X, train_X, dual_coef)
    args = concat_inputs(in_maps)
    outs = fn(*args)
    # per-core blocks come back as [P, NQC]; query q = qc*P + p
    blocks = np.asarray(outs[0]).reshape(N_CORES, P, NQC)
    out = blocks.transpose(0, 2, 1).reshape(-1)
    return out.astype(np.float32)


# revision 27
# speedup vs baseline: 1.0703x; 1.0703x over previous
"""RBF kernel ridge regression inference on 8 Trainium2 NeuronCores.

out[q] = sum_t exp(-gamma * ||X[q] - T[t]||^2) * coef[t]

Factored as sum_t (coef[t] * exp(-g*y2[t])) * exp(2g*dot[q,t] - g*x2[q]):
the train-side norm folds into the coefficients on the host, the query-side
norm folds into the ScalarE activation bias.  On device each core runs only
three pipelined stages per (query-chunk, train-segment) tile:

  TensorE : fp8e4 DoubleRow GEMM (queries stationary)  -> psum [128q, W]
  ScalarE : one wide Exp over the psum banks, bias=-g*x2 -> et bf16
  VectorE : et * coef' multiply-accumulate along free  -> partial out [128,1]

Train columns are processed in segments (ramping 1024 -> 2048 wide so the
first matmul only waits for ~1MB of DMA).  All device buffers are laid out
segment-major so every DMA is contiguous per partition (2-8KB descriptors).
Queries are sharded across the 8 cores; train data and coefficients are
replicated.
"""

import numpy as np
import ml_dtypes

GAMMA = 1.0
N_QUERY, N_TRAIN, D = 8192, 8192, 512
N_CORES = 8
P = 128
QPC = N_QUERY // N_CORES    # 1024 queries per core
KD = D // P                 # 4 contraction subtiles of 128
NQC = QPC // P              # 8 query chunks of 128 (stationary side)
TT = 512                    # one psum bank / one matmul free dim
SEGW = [512, 1536, 2048, 2048, 2048]    # train-column segment widths
SEGO = [sum(SEGW[:i]) for i in range(len(SEGW))]
NSEG = len(SEGW)

_CACHE = {}


def _build_program(probe=False):
    from contextlib import ExitStack

    import concourse.bass as bass
    import concourse.mybir as mybir
    import concourse.tile as tile
    from concourse import bacc

    f32 = mybir.dt.float32
    bf16 = mybir.dt.bfloat16
    fp8 = mybir.dt.float8e4
    AF = mybir.ActivationFunctionType
    MUL = mybir.AluOpType.mult
    ADD = mybir.AluOpType.add
    DR = mybir.MatmulPerfMode.DoubleRow

    nc = bacc.Bacc(
        "TRN2", target_bir_lowering=False, debug=False, num_devices=N_CORES
    )

    tt_d = nc.dram_tensor("tt_fp8", [P, KD * N_TRAIN], fp8, kind="ExternalInput").ap()
    xt_d = nc.dram_tensor("xt_fp8", [P, NQC * KD, P], fp8, kind="ExternalInput").ap()
    coef_d = nc.dram_tensor("coef_rep", [P, N_TRAIN], bf16, kind="ExternalInput").ap()
    x2_d = nc.dram_tensor("x2neg", [P, NQC], f32, kind="ExternalInput").ap()
    # natural [partition, qc] layout — host transposes; a "(c p) -> p c"
    # rearrange here would emit 1024 scattered 4B DMA writes (~11us tail)
    out_d = nc.dram_tensor("out", [P, NQC], f32, kind="ExternalOutput").ap()

    with tile.TileContext(nc) as tc, ExitStack() as ctx:
        res = ctx.enter_context(tc.tile_pool(name="res", bufs=1))
        etp = ctx.enter_context(tc.tile_pool(name="etp", bufs=3))
        scrp = ctx.enter_context(tc.tile_pool(name="scrp", bufs=2))
        psq = ctx.enter_context(tc.tile_pool(name="psq", bufs=2, space="PSUM"))

        # warm the ScalarE activation tables immediately: the implicit
        # ACT_TABLE_LOAD (~2.7us) binds before the first real Exp otherwise
        dum = res.tile([1, 1], f32, tag="dum")
        nc.vector.memset(dum[:], 0.0)
        dum2 = res.tile([1, 1], f32, tag="dum2")
        nc.scalar.activation(dum2[:], dum[:], AF.Exp)

        # ---- prologue: resident loads, segment-major so compute overlaps ----
        coef_sb = res.tile([P, N_TRAIN], bf16, tag="coef")
        xt_sb = res.tile([P, NQC * KD, P], fp8, tag="xt")
        half = NQC * KD // 2
        tts = []
        for s in range(NSEG):
            w = SEGW[s]
            t = res.tile([P, KD, w], fp8, tag=f"tt{s}", name=f"tt{s}")
            nc.sync.dma_start(
                t[:], tt_d[:, KD * SEGO[s] : KD * (SEGO[s] + w)].rearrange(
                    "p (k w) -> p k w", k=KD
                )
            )
            tts.append(t)
            if s == 0:
                nc.sync.dma_start(xt_sb[:, :KD, :], xt_d[:, :KD, :])
                nc.sync.dma_start(
                    coef_sb[:, : SEGW[0]], coef_d[:, : SEGW[0]]
                )
                x2c = res.tile([P, NQC], f32, tag="x2c")
                nc.sync.dma_start(x2c[:], x2_d[:])
                nc.sync.dma_start(xt_sb[:, KD:half, :], xt_d[:, KD:half, :])
                nc.sync.dma_start(xt_sb[:, half:, :], xt_d[:, half:, :])
            else:
                nc.sync.dma_start(
                    coef_sb[:, SEGO[s] : SEGO[s] + w],
                    coef_d[:, SEGO[s] : SEGO[s] + w],
                )

        pacc = res.tile([P, NQC * NSEG], f32, tag="pacc")
        outc = res.tile([P, NQC], f32, tag="outc")

        # warm the PE's HAM clock gate (3.4us of sustained activity lifts it
        # from 1.2 to 2.4GHz) with two fp32 dummy matmuls (4 cyc/row) while
        # the first DMAs are still in flight; real matmuls then start warm
        dmov = res.tile([1, 512], f32, tag="dmov")
        nc.vector.memset(dmov[:], 0.0)
        ps_warm = psq.tile([P, 4 * SEGW[0]], f32, tag="ps", name="ps_warm")
        for i in range(2):
            nc.tensor.matmul(
                ps_warm[0:1, i * 512 : (i + 1) * 512], dum[:], dmov[:],
                start=True, stop=True,
            )

        # ---- main loop: train segments x 8 query chunks ----
        for s in range(NSEG):
            w = SEGW[s]
            for qc in range(NQC):
                ps = psq.tile([P, w], f32, tag="ps")
                for kd2 in range(KD // 2):
                    wst = xt_sb[:, qc * KD + 2 * kd2 : qc * KD + 2 * kd2 + 2, :]
                    for k in range(w // TT):
                        nc.tensor.matmul(
                            ps[:, k * TT : (k + 1) * TT],
                            wst,
                            tts[s][:, 2 * kd2 : 2 * kd2 + 2, k * TT : (k + 1) * TT],
                            start=(kd2 == 0),
                            stop=(kd2 == KD // 2 - 1),
                            perf_mode=DR,
                        )
                et = etp.tile([P, w], bf16, tag="et")
                nc.scalar.activation(
                    et[:], ps[:], AF.Exp, bias=x2c[:, qc : qc + 1], scale=2.0 * GAMMA
                )
                scr = scrp.tile([P, w], bf16, tag="scr")
                nc.vector.scalar_tensor_tensor(
                    scr[:], et[:], 1.0, coef_sb[:, SEGO[s] : SEGO[s] + w], MUL, MUL,
                    accum_out=pacc[:, qc * NSEG + s : qc * NSEG + s + 1],
                )
                if s == NSEG - 1:
                    # all partials for this qc are in; fold them right away
                    nc.vector.tensor_reduce(
                        outc[:, qc : qc + 1],
                        pacc[:, qc * NSEG : (qc + 1) * NSEG],
                        mybir.AxisListType.X,
                        ADD,
                    )
        nc.sync.dma_start(out_d, outc[:])

        if probe:
            # one-shot DVE perf-mode probes (read from the trace, then drop)
            prb_mul = res.tile([P, 2048], bf16, tag="prbm")
            nc.vector.tensor_mul(
                prb_mul[:], coef_sb[:, 0:2048], coef_sb[:, 2048:4096]
            )
            prb_ts = res.tile([P, 2048], bf16, tag="prbt")
            prb_acc = res.tile([P, 1], f32, tag="prba")
            nc.vector.tensor_scalar(
                prb_ts[:], prb_mul[:], 1.0, 0.0, MUL, ADD, accum_out=prb_acc[:]
            )

    nc.compile()
    return nc


def _get_program():
    if "nc" not in _CACHE:
        _CACHE["nc"] = _build_program()
    return _CACHE["nc"]


def make_in_maps(X, train_X, dual_coef):
    fp8 = ml_dtypes.float8_e4m3
    bf = ml_dtypes.bfloat16

    # train side, segment-major: per segment [P, KD*w] contiguous
    segs = []
    for s in range(NSEG):
        t0, w = SEGO[s], SEGW[s]
        seg = train_X[t0 : t0 + w].T.reshape(KD, P, w).transpose(1, 0, 2)
        segs.append(seg.reshape(P, KD * w))
    ttb = np.ascontiguousarray(np.concatenate(segs, axis=1)).astype(fp8)

    # fold exp(-g*||T_t||^2) into the coefficients, replicate across partitions
    y2 = np.sum(train_X.astype(np.float32) ** 2, axis=1)
    coef_f = (dual_coef.astype(np.float32) * np.exp(-GAMMA * y2)).astype(bf)
    coefb = np.ascontiguousarray(np.broadcast_to(coef_f[None, :], (P, N_TRAIN)))

    in_maps = []
    for c in range(N_CORES):
        Xc = X[c * QPC : (c + 1) * QPC]
        # [p, qc*KD+kd, j] = Xc[qc*128+j, kd*128+p]
        xtb = np.ascontiguousarray(
            Xc.reshape(NQC, P, KD, P).transpose(3, 0, 2, 1).reshape(P, NQC * KD, P)
        ).astype(fp8)
        x2 = np.sum(Xc.astype(np.float32) ** 2, axis=1)
        x2neg = np.ascontiguousarray((-GAMMA * x2).reshape(NQC, P).T)
        in_maps.append(
            {
                "tt_fp8": ttb,
                "xt_fp8": xtb,
                "coef_rep": coefb,
                "x2neg": x2neg,
            }
        )
    return in_maps


def _get_callable():
    """Cached (fn, in_names, out_names, out_avals, zero_outs, mesh) for the
    sharded 8-core NEFF execution."""
    if "call" in _CACHE:
        return _CACHE["call"]

    import jax
    from jax.sharding import Mesh, PartitionSpec
    from jax.experimental.shard_map import shard_map

    import concourse.mybir as mybir
    from concourse import bass2jax
    from concourse.bass2jax import install_neuronx_cc_hook

    install_neuronx_cc_hook()
    nc = _get_program()

    partition_name = (
        nc.partition_id_tensor.name if nc.partition_id_tensor else None
    )
    in_names, out_names, out_avals, zero_outs = [], [], [], []
    for alloc in nc.m.functions[0].allocations:
        if not isinstance(alloc, mybir.MemoryLocationSet):
            continue
        if alloc.kind not in ("ExternalInput", "ExternalOutput"):
            continue
        name = alloc.memorylocations[0].name
        if alloc.kind == "ExternalInput":
            if name != partition_name:
                in_names.append(name)
        else:
            out_names.append(name)
            shape = tuple(alloc.tensor_shape)
            dtype = mybir.dt.np(alloc.dtype)
            out_avals.append(jax.core.ShapedArray(shape, dtype))
            zero_outs.append(np.zeros(shape, dtype))
    all_in_names = in_names + out_names
    if partition_name is not None:
        all_in_names = all_in_names + [partition_name]

    def _body(*args):
        operands = list(args)
        if partition_name is not None:
            operands.append(bass2jax.partition_id_tensor())
        outs = bass2jax._bass_exec_p.bind(
            *operands,
            out_avals=tuple(out_avals),
            in_names=tuple(all_in_names),
            out_names=tuple(out_names),
            lowering_input_output_aliases=(),
            sim_require_finite=True,
            sim_require_nnan=True,
            nc=nc,
        )
        return tuple(outs)

    devices = jax.devices()[:N_CORES]
    mesh = Mesh(np.asarray(devices), ("core",))
    n_all = len(in_names) + len(out_names)
    fn = jax.jit(
        shard_map(
            _body,
            mesh=mesh,
            in_specs=(PartitionSpec("core"),) * n_all,
            out_specs=(PartitionSpec("core"),) * len(out_names),
            check_rep=False,
        ),
        keep_unused=True,
    )
    _CACHE["call"] = (fn, in_names, out_names, out_avals, zero_outs, mesh)
    return _CACHE["call"]


def concat_inputs(in_maps):
    fn, in_names, out_names, out_avals, zero_outs, mesh = _get_callable()
    concat_in = [
        np.concatenate([np.asarray(m[name]) for m in in_maps], axis=0)
        for name in in_names
    ]
    concat_zeros = [
        np.zeros((N_CORES * z.shape[0], *z.shape[1:]), z.dtype) for z in zero_outs
    ]
    return concat_in + concat_zeros


def kernel(X, train_X, dual_coef):
    X = np.asarray(X, dtype=np.float32)
    train_X = np.asarray(train_X, dtype=np.float32)
    dual_coef = np.asarray(dual_coef, dtype=np.float32)

    fn, in_names, out_names, out_avals, zero_outs, mesh = _get_callable()
    in_maps = make_in_maps(X, train_X, dual_coef)
    args = concat_inputs(in_maps)
    outs = fn(*args)
    # per-core blocks come back as [P, NQC]; query q = qc*P + p
    blocks = np.asarray(outs[0]).reshape(N_CORES, P, NQC)
    out = blocks.transpose(0, 2, 1).reshape(-1)
    return out.astype(np.float32)
